# revision 11
# baseline (speedup 1.0000x reference)
"""DualStreamEncoderAttention Trainium2 kernel (v2).

Sharding: 8 cores = 4 samples x 2 head-groups (8 heads each). Each core
computes, for its sample, both streams' QKV(+RoPE) for its 8 heads,
cross-stream attention (KV concat is per-sample, head sharding is clean),
and a partial out-projection over its heads' rows of Wout. The host sums
the two partial projections per sample. No collectives; pure SPMD.

Speed strategy (S=1024, H=1024, D=64, 8 local heads):
  - LayerNorm is folded on the host: the kernel receives
    xhat = r*(x-mu) pre-transposed to [H, S] (r,mu are pure input
    functions), plus gamma folded into W and a rank-1 beta@W correction
    row added via a K=1 matmul into the same PSUM accumulation.
  - Q,K are produced in a DoubleRow layout: per (stream, ht) a tile
    [128 = 4 heads x 32 dlow, 2 d-halves, S] in fp8e4. QK^T scores run
    as fp8 DoubleRow matmuls (0.5 cycles/col, contraction 2x32=64 on 32
    partitions) - 2x the bf16 PE rate.
  - RoPE's rotate-half partner (d <-> d+32) lives on the same partition
    in the other d-half plane, so the rotation is plain elementwise math
    between the lo/hi projection halves - no partition-shuffle DMAs.
    Sin tables are pre-negated on the host; the final add emits fp8.
  - exp on the Scalar engine from PSUM ([128,1024] tiles, scale=1/8
    folded in), output bf16; softmax denominator via a ones-column in V
    (PV accumulator row 64). PV is bf16.
  - Emission order lets the Tile scheduler overlap everything: stream-a
    prep -> attention over stream-a keys starts (~20us in) while
    stream-b prep runs on PE/DVE. Early slots bridge the a->b key gap by
    draining partial PV accumulators to SBUF. The Activation engine
    (256 x [128,1024] exp = the hard floor) stays ~saturated; all PSUM
    evictions are on DVE/Pool.
"""

import sys

for _p in ("/opt/trn_rl_repo", "/root/.axon_site/_ro/trn_rl_repo"):
    if _p not in sys.path:
        sys.path.insert(0, _p)

import numpy as np

S = 1024
H = 1024
NH = 16
D = 64
NHL = 8          # heads per core
P = 128
N_CORES = 8
LN_EPS = 1e-5
ROPE_BASE = 10000.0
SCALE = float(D) ** -0.5

# leading attention slots split PV accumulation into stream-a / stream-b
# phases (bridges the stream-b prep gap)
N_SPLIT = 6

_PROGRAM = None


def _rope_tables(height, width, head_dim=D):
    """Mirror of reference.rope_2d_tables in numpy float32."""
    height = int(height)
    width = int(width)
    dim_x = head_dim // 2
    dim_y = head_dim - dim_x
    inv_fx = 1.0 / (ROPE_BASE ** (np.arange(0, dim_x, 2, dtype=np.float32) / np.float32(dim_x)))
    inv_fy = 1.0 / (ROPE_BASE ** (np.arange(0, dim_y, 2, dtype=np.float32) / np.float32(dim_y)))
    fx = np.arange(width, dtype=np.float32)[:, None] * inv_fx[None, :]
    fy = np.arange(height, dtype=np.float32)[:, None] * inv_fy[None, :]
    fx = np.concatenate([fx, fx], axis=-1)  # [W, dim_x]
    fy = np.concatenate([fy, fy], axis=-1)  # [H, dim_y]
    cos = np.concatenate([
        np.broadcast_to(np.cos(fx)[None, :, :], (height, width, dim_x)),
        np.broadcast_to(np.cos(fy)[:, None, :], (height, width, dim_y)),
    ], axis=-1).reshape(height * width, head_dim).astype(np.float32)
    sin = np.concatenate([
        np.broadcast_to(np.sin(fx)[None, :, :], (height, width, dim_x)),
        np.broadcast_to(np.sin(fy)[:, None, :], (height, width, dim_y)),
    ], axis=-1).reshape(height * width, head_dim).astype(np.float32)
    return cos, sin


def _build_program(do_compile=True):
    import concourse.mybir as mybir
    import concourse.tile as tile
    from concourse import bacc

    f32 = mybir.dt.float32
    f32r = mybir.dt.float32r
    bf16 = mybir.dt.bfloat16
    fp8 = mybir.dt.float8e4
    DR = mybir.MatmulPerfMode.DoubleRow
    AF = mybir.ActivationFunctionType

    nc = bacc.Bacc("TRN2")

    # ---- DRAM parameters (per-core tensors; same program on all cores) ----
    # xhat: r*(x-mu), pre-transposed [H, S]
    xh_d = [nc.dram_tensor(f"xh_s{s}", [H, S], f32r, kind="ExternalInput") for s in range(2)]
    # qk weights (gamma-folded), columns in DR order: 8 blocks of 128 = (q|k, ht, half)
    wqk_d = [nc.dram_tensor(f"wqk_s{s}", [H, 2 * NHL * D], f32r, kind="ExternalInput") for s in range(2)]
    wv_d = [nc.dram_tensor(f"wv_s{s}", [H, NHL * D], f32r, kind="ExternalInput") for s in range(2)]
    cqk_d = [nc.dram_tensor(f"cqk_s{s}", [P, 8], f32, kind="ExternalInput") for s in range(2)]
    wout_d = [nc.dram_tensor(f"wout_s{s}", [NHL * D, H], f32r, kind="ExternalInput") for s in range(2)]
    cos_d = nc.dram_tensor("cosdr", [P, 2 * S], bf16, kind="ExternalInput")
    sin_d = nc.dram_tensor("sindr", [P, 2 * S], bf16, kind="ExternalInput")  # pre-negated i=0 half
    out_d = [nc.dram_tensor(f"out_s{s}", [S, H], f32, kind="ExternalOutput") for s in range(2)]

    with tile.TileContext(nc) as tc:
        with (
            tc.tile_pool(name="consts", bufs=1) as consts,
            tc.tile_pool(name="persist", bufs=1) as persist,
            tc.tile_pool(name="attx", bufs=1) as attx,
            tc.tile_pool(name="small", bufs=2) as small,
            tc.tile_pool(name="pp", bufs=2, space="PSUM") as pp,
            tc.tile_pool(name="scps", bufs=1, space="PSUM") as scps,
            tc.tile_pool(name="accps", bufs=2, space="PSUM") as accps,
        ):
            cosdr = consts.tile([P, 2, S], bf16, tag="cosdr")
            nc.sync.dma_start(out=cosdr, in_=cos_d[:].rearrange("p (i s) -> p i s", i=2))
            sindr = consts.tile([P, 2, S], bf16, tag="sindr")
            nc.sync.dma_start(out=sindr, in_=sin_d[:].rearrange("p (i s) -> p i s", i=2))
            zeroc = consts.tile([P, 1], f32, tag="zeroc")
            nc.vector.memset(zeroc, 0.0)

            # persistent per-stream state; q/k as head-pair tiles
            # [64 = 2 heads x 32 dlow, 2 d-halves, S] fp8
            qdr = [[persist.tile([64, 2, S], fp8, tag=f"qdr{s}_{pt}", name=f"qdr{s}_{pt}")
                    for pt in range(4)] for s in range(2)]
            kdr = [[persist.tile([64, 2, S], fp8, tag=f"kdr{s}_{pt}", name=f"kdr{s}_{pt}")
                    for pt in range(4)] for s in range(2)]
            v_sb = [[persist.tile([P, NHL, D + 1], bf16, tag=f"v{s}_{st}", name=f"v{s}_{st}")
                     for st in range(8)] for s in range(2)]
            attn = [[persist.tile([P, S], f32r, tag=f"attn{s}_{p}", name=f"attn{s}_{p}")
                     for p in range(4)] for s in range(2)]

            es_pool = attx
            pa_pool = attx

            # ---------------- prep per stream ----------------
            def prep(s):
                with tc.tile_pool(name=f"prep{s}", bufs=1) as prep_p:
                    xh = [prep_p.tile([P, S], f32r, tag=f"xh{hc}", name=f"xh{hc}") for hc in range(8)]
                    for hc in range(8):
                        (nc.sync if hc % 2 == 0 else nc.gpsimd).dma_start(
                            out=xh[hc], in_=xh_d[s][hc * P:(hc + 1) * P, :])
                    cqk_sb = prep_p.tile([P, 8], f32, tag="cqk", name="cqk_sb")
                    nc.sync.dma_start(out=cqk_sb, in_=cqk_d[s][:])

                    # q/k projections in DR block order; q (blocks 0-3) first
                    for qk in range(2):
                        dst = (qdr if qk == 0 else kdr)[s]
                        for ht in range(2):
                            stg_t = [[None, None], [None, None]]  # [half][sc]
                            for half in range(2):
                                b = qk * 4 + ht * 2 + half
                                wqf = prep_p.tile([P, 8, P], f32r, tag="wqf", bufs=2, name="wqf")
                                nc.sync.dma_start(
                                    out=wqf,
                                    in_=wqk_d[s][:, b * P:(b + 1) * P].rearrange("(c p) n -> p c n", p=P))
                                for sc in range(2):
                                    psq = pp.tile([P, 512], f32, tag="mm", name="psq")
                                    for kc in range(8):
                                        nc.tensor.matmul(
                                            psq,
                                            wqf[:, kc, :],
                                            xh[kc][:, sc * 512:(sc + 1) * 512],
                                            start=(kc == 0), stop=(kc == 7),
                                        )
                                    stg = prep_p.tile([P, 512], bf16, tag="stg", bufs=5, name="stg")
                                    nc.vector.tensor_copy(out=stg, in_=psq)
                                    stg_t[half][sc] = stg
                            # rope: out half i mixes staged lo/hi planes; the
                            # add splits per head-pair tile (2ht+g)
                            for i in range(2):
                                b_i = qk * 4 + ht * 2 + i
                                b_o = qk * 4 + ht * 2 + (1 - i)
                                for sc in range(2):
                                    csl = slice(sc * 512, (sc + 1) * 512)
                                    tmp = small.tile([P, 512], bf16, tag="rtmp", bufs=3, name="rtmp")
                                    nc.vector.scalar_tensor_tensor(
                                        tmp, stg_t[1 - i][sc], cqk_sb[:, b_o:b_o + 1], sindr[:, i, csl],
                                        op0=mybir.AluOpType.add, op1=mybir.AluOpType.mult)
                                    qc = small.tile([P, 512], bf16, tag="rqc", bufs=3, name="rqc")
                                    nc.vector.scalar_tensor_tensor(
                                        qc, stg_t[i][sc], cqk_sb[:, b_i:b_i + 1], cosdr[:, i, csl],
                                        op0=mybir.AluOpType.add, op1=mybir.AluOpType.mult)
                                    for g in range(2):
                                        nc.vector.tensor_add(
                                            dst[2 * ht + g][:, i, csl],
                                            tmp[64 * g:64 * g + 64, :], qc[64 * g:64 * g + 64, :])

                    # V natural [s, n] + ones column, bf16
                    wvf = prep_p.tile([P, 8, NHL * D], f32r, tag="wvf", name="wvf")
                    nc.sync.dma_start(out=wvf, in_=wv_d[s][:].rearrange("(c p) n -> p c n", p=P))
                    for st in range(8):
                        psv = pp.tile([P, 512], f32, tag="mm", name="psv")
                        for kc in range(8):
                            nc.tensor.matmul(
                                psv,
                                xh[kc][:, st * P:(st + 1) * P],
                                wvf[:, kc, :],
                                start=(kc == 0), stop=(kc == 7),
                            )
                        nc.vector.memset(v_sb[s][st][:, :, D:D + 1], 1.0)
                        nc.vector.tensor_copy(
                            out=v_sb[s][st][:, :, 0:D],
                            in_=psv.rearrange("p (h d) -> p h d", d=D),
                        )

            # ---------------- attention slot ----------------
            def slot(s, ht, hh, split):
                h = 4 * ht + hh
                pt = h // 2
                pr = slice(32 * (h % 2), 32 * (h % 2) + 32)
                row = (h % 2) * 64
                pair = h // 2

                def half_attn(acc, ts, start, stop):
                    for tst in range(8):
                        ps = scps.tile([P, S], f32, tag="sc", name="ps")
                        for sc in range(2):
                            csl = slice(sc * 512, (sc + 1) * 512)
                            nc.tensor.matmul(
                                ps[:, csl],
                                kdr[ts][pt][pr, :, tst * P:(tst + 1) * P],
                                qdr[s][pt][pr, :, csl],
                                perf_mode=DR,
                            )
                        es = es_pool.tile([P, S], bf16, tag="es", bufs=4, name="es")
                        nc.scalar.activation(out=es, in_=ps, func=AF.Exp, bias=zeroc, scale=SCALE)
                        for sc in range(2):
                            csl = slice(sc * 512, (sc + 1) * 512)
                            nc.tensor.matmul(
                                acc[:, csl],
                                v_sb[ts][tst][:, h, :],
                                es[:, csl],
                                start=(start and tst == 0), stop=(stop and tst == 7),
                            )

                def normalize(src):
                    rstg = small.tile([1, S], f32, tag="rstg", bufs=1, name="rstg")
                    nc.vector.reciprocal(out=rstg, in_=src[D:D + 1, :])
                    rbc = small.tile([D, S], f32, tag="rbc", bufs=1, name="rbc")
                    nc.gpsimd.partition_broadcast(rbc, rstg)
                    nc.vector.tensor_mul(attn[s][pair][row:row + D, :], src[0:D, :], rbc)

                if split:
                    acc = accps.tile([D + 1, S], f32, tag="acc", name="acc")
                    half_attn(acc, 0, True, True)
                    pa = pa_pool.tile([D + 1, S], f32, tag="pa", bufs=N_SPLIT, name="pa")
                    nc.vector.tensor_copy(out=pa, in_=acc)
                    acc = accps.tile([D + 1, S], f32, tag="acc", name="acc")
                    half_attn(acc, 1, True, True)
                    nsum = small.tile([D + 1, S], f32, tag="nsum", bufs=1, name="nsum")
                    nc.vector.tensor_add(nsum, acc, pa)
                    normalize(nsum)
                else:
                    acc = accps.tile([D + 1, S], f32, tag="acc", name="acc")
                    half_attn(acc, 0, True, False)
                    half_attn(acc, 1, False, True)
                    normalize(acc)

            # ---------------- tail: out-projection per stream ----------------
            def tail(s):
                with tc.tile_pool(name=f"wo{s}", bufs=1) as wop:
                    wo_t = [wop.tile([P, H], f32r, tag=f"wo{p}", name=f"wo{p}") for p in range(4)]
                    for p in range(4):
                        nc.sync.dma_start(out=wo_t[p], in_=wout_d[s][p * P:(p + 1) * P, :])
                    for st in range(8):
                        for oc in range(2):
                            pso = pp.tile([P, 512], f32, tag="mm", name="pso")
                            for p in range(4):
                                nc.tensor.matmul(
                                    pso,
                                    attn[s][p][:, st * P:(st + 1) * P],
                                    wo_t[p][:, oc * 512:(oc + 1) * 512],
                                    start=(p == 0), stop=(p == 3),
                                )
                            osb = small.tile([P, 512], f32, tag="osb", bufs=3, name="osb")
                            if s == 1 and (st + oc) % 2 == 0:
                                nc.scalar.copy(out=osb, in_=pso)
                            else:
                                nc.vector.tensor_copy(out=osb, in_=pso)
                            (nc.gpsimd if (st + oc) % 2 == 0 else nc.sync).dma_start(
                                out=out_d[s][st * P:(st + 1) * P, oc * 512:(oc + 1) * 512], in_=osb)

            # ---------------- emission ----------------
            prep(0)
            prep(1)
            idx = 0
            for s in range(2):
                for ht in range(2):
                    for hh in range(4):
                        slot(s, ht, hh, split=(idx < N_SPLIT))
                        idx += 1
                tail(s)

    if do_compile:
        nc.compile()
    return nc


def _host_prep(x_a, x_b, Wqkv_a, Wqkv_b, Wout_a, Wout_b,
               gamma_a, beta_a, gamma_b, beta_b, height, width):
    """Build the 8 per-core input maps."""
    import ml_dtypes
    cos, sin = _rope_tables(height, width)      # [S, 64]

    # DR rope tables [128, 2, S]: partition p = 32*hh + dl (repeats over hh)
    dl = np.arange(32)
    cos_dr = np.empty((P, 2, S), np.float32)
    sin_dr = np.empty((P, 2, S), np.float32)
    for hh in range(4):
        rows = 32 * hh + dl
        cos_dr[rows, 0, :] = cos[:, dl].T          # cos[s, dl]
        cos_dr[rows, 1, :] = cos[:, 32 + dl].T
        sin_dr[rows, 0, :] = -sin[:, dl].T         # pre-negated for i=0
        sin_dr[rows, 1, :] = sin[:, 32 + dl].T
    cos_dr = np.ascontiguousarray(cos_dr.reshape(P, 2 * S).astype(ml_dtypes.bfloat16))
    sin_dr = np.ascontiguousarray(sin_dr.reshape(P, 2 * S).astype(ml_dtypes.bfloat16))

    # host LayerNorm fold: xhat = r*(x-mu), shipped transposed [H, S]
    def xhat(x):
        x = x.astype(np.float32)
        mu = x.mean(axis=-1, keepdims=True)
        var = ((x - mu) ** 2).mean(axis=-1, keepdims=True)
        r = 1.0 / np.sqrt(var + LN_EPS)
        return ((x - mu) * r).astype(np.float32)

    streams = []
    vshifts = []
    for (W, Wo, g, b) in ((Wqkv_a, Wout_a, gamma_a, beta_a), (Wqkv_b, Wout_b, gamma_b, beta_b)):
        Wg = (W * g[:, None]).astype(np.float32)       # gamma-folded
        cfull = (b.astype(np.float64) @ W.astype(np.float64)).astype(np.float32)  # beta@W [3H]
        W4 = Wg.reshape(H, 3, NH, D)
        c4 = cfull.reshape(3, NH, D)
        per_hg = []
        for hg in range(2):
            h0 = hg * NHL
            # DR column order: blocks (qk, ht, half) of 128 cols = (hh, dl)
            cols = []
            ccols = []
            for qk in range(2):
                for ht in range(2):
                    for half in range(2):
                        for hh in range(4):
                            head = h0 + 4 * ht + hh
                            dsl = slice(32 * half, 32 * half + 32)
                            cols.append(W4[:, qk, head, dsl])      # [H, 32]
                            ccols.append(c4[qk, head, dsl])        # [32]
            wqk = np.ascontiguousarray(np.concatenate(cols, axis=1))       # [H, 1024]
            # beta@W per qk column as per-partition scalars [128, 8 blocks]
            cqk_blk = np.ascontiguousarray(
                np.concatenate(ccols).reshape(8, P).T.astype(np.float32))   # [128, 8]
            wv = np.ascontiguousarray(W4[:, 2, h0:h0 + NHL, :].reshape(H, NHL * D))
            wout = np.ascontiguousarray(Wo.reshape(NH, D, H)[h0:h0 + NHL].reshape(NHL * D, H).astype(np.float32))
            per_hg.append(dict(wqk=wqk, wv=wv, cqk=cqk_blk, wout=wout))
        # exact host-side V correction: beta@Wv shifts attn uniformly
        # (softmax weights sum to 1), so it lands as a constant row on out
        vshift = (cfull[2 * H:3 * H].astype(np.float64) @ Wo.astype(np.float64)).astype(np.float32)
        streams.append(per_hg)
        vshifts.append(vshift)

    in_maps = []
    B = x_a.shape[0]
    xh_a = [np.ascontiguousarray(xhat(x_a[b_i]).T) for b_i in range(B)]
    xh_b = [np.ascontiguousarray(xhat(x_b[b_i]).T) for b_i in range(B)]
    for c in range(N_CORES):
        b_i, hg = (c // 2) % B, c % 2
        m = {
            "xh_s0": xh_a[b_i],
            "xh_s1": xh_b[b_i],
            "cosdr": cos_dr, "sindr": sin_dr,
        }
        for s in range(2):
            blk = streams[s][hg]
            m[f"wqk_s{s}"] = blk["wqk"]
            m[f"wv_s{s}"] = blk["wv"]
            m[f"cqk_s{s}"] = blk["cqk"]
            m[f"wout_s{s}"] = blk["wout"]
        in_maps.append(m)
    return in_maps, vshifts


def kernel(x_a, x_b, Wqkv_a, Wqkv_b, Wout_a, Wout_b,
           gamma_a, beta_a, gamma_b, beta_b, height, width):
    from concourse.bass_utils import run_bass_kernel_spmd

    x_a = np.asarray(x_a, dtype=np.float32)
    x_b = np.asarray(x_b, dtype=np.float32)
    B = x_a.shape[0]
    in_maps, vshifts = _host_prep(x_a, x_b,
                         np.asarray(Wqkv_a, np.float32), np.asarray(Wqkv_b, np.float32),
                         np.asarray(Wout_a, np.float32), np.asarray(Wout_b, np.float32),
                         np.asarray(gamma_a, np.float32), np.asarray(beta_a, np.float32),
                         np.asarray(gamma_b, np.float32), np.asarray(beta_b, np.float32),
                         height, width)
    nc = _get_program()
    res = run_bass_kernel_spmd(nc, in_maps, list(range(N_CORES))).results
    out_a = np.empty((B, S, H), np.float32)
    out_b = np.empty((B, S, H), np.float32)
    for b_i in range(B):
        out_a[b_i] = res[2 * b_i]["out_s0"] + res[2 * b_i + 1]["out_s0"] + vshifts[0]
        out_b[b_i] = res[2 * b_i]["out_s1"] + res[2 * b_i + 1]["out_s1"] + vshifts[1]
    return out_a, out_b


def _get_program():
    global _PROGRAM
    if _PROGRAM is None:
        _PROGRAM = _build_program()
    return _PROGRAM


# revision 12
# speedup vs baseline: 1.5161x; 1.5161x over previous
"""DualStreamEncoderAttention Trainium2 kernel (v2).

Sharding: 8 cores = 4 samples x 2 head-groups (8 heads each). Each core
computes, for its sample, both streams' QKV(+RoPE) for its 8 heads,
cross-stream attention (KV concat is per-sample, head sharding is clean),
and a partial out-projection over its heads' rows of Wout. The host sums
the two partial projections per sample. No collectives; pure SPMD.

Speed strategy (S=1024, H=1024, D=64, 8 local heads):
  - LayerNorm is folded on the host: the kernel receives
    xhat = r*(x-mu) pre-transposed to [H, S] (r,mu are pure input
    functions), plus gamma folded into W and a rank-1 beta@W correction
    row added via a K=1 matmul into the same PSUM accumulation.
  - Q,K are produced in a DoubleRow layout: per (stream, ht) a tile
    [128 = 4 heads x 32 dlow, 2 d-halves, S] in fp8e4. QK^T scores run
    as fp8 DoubleRow matmuls (0.5 cycles/col, contraction 2x32=64 on 32
    partitions) - 2x the bf16 PE rate.
  - RoPE's rotate-half partner (d <-> d+32) lives on the same partition
    in the other d-half plane, so the rotation is plain elementwise math
    between the lo/hi projection halves - no partition-shuffle DMAs.
    Sin tables are pre-negated on the host; the final add emits fp8.
  - exp on the Scalar engine from PSUM ([128,1024] tiles, scale=1/8
    folded in), output bf16; softmax denominator via a ones-column in V
    (PV accumulator row 64). PV is bf16.
  - Emission order lets the Tile scheduler overlap everything: stream-a
    prep -> attention over stream-a keys starts (~20us in) while
    stream-b prep runs on PE/DVE. Early slots bridge the a->b key gap by
    draining partial PV accumulators to SBUF. The Activation engine
    (256 x [128,1024] exp = the hard floor) stays ~saturated; all PSUM
    evictions are on DVE/Pool.
"""

import sys

for _p in ("/opt/trn_rl_repo", "/root/.axon_site/_ro/trn_rl_repo"):
    if _p not in sys.path:
        sys.path.insert(0, _p)

import numpy as np

S = 1024
H = 1024
NH = 16
D = 64
NHL = 8          # heads per core
P = 128
N_CORES = 8
LN_EPS = 1e-5
ROPE_BASE = 10000.0
SCALE = float(D) ** -0.5

# leading attention slots split PV accumulation into stream-a / stream-b
# phases (bridges the stream-b prep gap)
N_SPLIT = 6

_PROGRAM = None


def _rope_tables(height, width, head_dim=D):
    """Mirror of reference.rope_2d_tables in numpy float32."""
    height = int(height)
    width = int(width)
    dim_x = head_dim // 2
    dim_y = head_dim - dim_x
    inv_fx = 1.0 / (ROPE_BASE ** (np.arange(0, dim_x, 2, dtype=np.float32) / np.float32(dim_x)))
    inv_fy = 1.0 / (ROPE_BASE ** (np.arange(0, dim_y, 2, dtype=np.float32) / np.float32(dim_y)))
    fx = np.arange(width, dtype=np.float32)[:, None] * inv_fx[None, :]
    fy = np.arange(height, dtype=np.float32)[:, None] * inv_fy[None, :]
    fx = np.concatenate([fx, fx], axis=-1)  # [W, dim_x]
    fy = np.concatenate([fy, fy], axis=-1)  # [H, dim_y]
    cos = np.concatenate([
        np.broadcast_to(np.cos(fx)[None, :, :], (height, width, dim_x)),
        np.broadcast_to(np.cos(fy)[:, None, :], (height, width, dim_y)),
    ], axis=-1).reshape(height * width, head_dim).astype(np.float32)
    sin = np.concatenate([
        np.broadcast_to(np.sin(fx)[None, :, :], (height, width, dim_x)),
        np.broadcast_to(np.sin(fy)[:, None, :], (height, width, dim_y)),
    ], axis=-1).reshape(height * width, head_dim).astype(np.float32)
    return cos, sin


def _build_program(do_compile=True):
    import concourse.mybir as mybir
    import concourse.tile as tile
    from concourse import bacc

    f32 = mybir.dt.float32
    f32r = mybir.dt.float32r
    bf16 = mybir.dt.bfloat16
    fp8 = mybir.dt.float8e4
    DR = mybir.MatmulPerfMode.DoubleRow
    AF = mybir.ActivationFunctionType

    nc = bacc.Bacc("TRN2")

    # ---- DRAM parameters (per-core tensors; same program on all cores) ----
    # xhat: r*(x-mu), pre-transposed [H, S]
    xh_d = [nc.dram_tensor(f"xh_s{s}", [H, S], f32r, kind="ExternalInput") for s in range(2)]
    # qk weights (gamma-folded), columns in DR order: 8 blocks of 128 = (q|k, ht, half)
    wqk_d = [nc.dram_tensor(f"wqk_s{s}", [H, 2 * NHL * D], f32r, kind="ExternalInput") for s in range(2)]
    wv_d = [nc.dram_tensor(f"wv_s{s}", [H, NHL * D], f32r, kind="ExternalInput") for s in range(2)]
    cqk_d = [nc.dram_tensor(f"cqk_s{s}", [P, 8], f32, kind="ExternalInput") for s in range(2)]
    wout_d = [nc.dram_tensor(f"wout_s{s}", [NHL * D, H], f32r, kind="ExternalInput") for s in range(2)]
    cos_d = nc.dram_tensor("cosdr", [P, 2 * S], bf16, kind="ExternalInput")
    sin_d = nc.dram_tensor("sindr", [P, 2 * S], bf16, kind="ExternalInput")  # pre-negated i=0 half
    out_d = [nc.dram_tensor(f"out_s{s}", [S, H], f32, kind="ExternalOutput") for s in range(2)]

    with tile.TileContext(nc) as tc:
        with (
            tc.tile_pool(name="consts", bufs=1) as consts,
            tc.tile_pool(name="persist", bufs=1) as persist,
            tc.tile_pool(name="attx", bufs=1) as attx,
            tc.tile_pool(name="small", bufs=2) as small,
            tc.tile_pool(name="pp", bufs=2, space="PSUM") as pp,
            tc.tile_pool(name="scps", bufs=2, space="PSUM") as scps,
            tc.tile_pool(name="accps", bufs=1, space="PSUM") as accps,
        ):
            cosdr = consts.tile([P, 2, S], bf16, tag="cosdr")
            nc.sync.dma_start(out=cosdr, in_=cos_d[:].rearrange("p (i s) -> p i s", i=2))
            sindr = consts.tile([P, 2, S], bf16, tag="sindr")
            nc.sync.dma_start(out=sindr, in_=sin_d[:].rearrange("p (i s) -> p i s", i=2))
            zeroc = consts.tile([P, 1], f32, tag="zeroc")
            nc.vector.memset(zeroc, 0.0)

            # persistent per-stream state; q/k as 4-head DR tiles
            # [128 = 4 heads x 32 dlow, 2 d-halves, S] fp8
            qdr = [[persist.tile([P, 2, S], fp8, tag=f"qdr{s}_{ht}", name=f"qdr{s}_{ht}")
                    for ht in range(2)] for s in range(2)]
            kdr = [[persist.tile([P, 2, S], fp8, tag=f"kdr{s}_{ht}", name=f"kdr{s}_{ht}")
                    for ht in range(2)] for s in range(2)]
            v_sb = [[persist.tile([P, NHL, D + 1], bf16, tag=f"v{s}_{st}", name=f"v{s}_{st}")
                     for st in range(8)] for s in range(2)]
            attn = [[persist.tile([P, S], f32r, tag=f"attn{s}_{p}", name=f"attn{s}_{p}")
                     for p in range(4)] for s in range(2)]

            es_pool = attx
            pa_pool = attx

            # ---------------- prep per stream ----------------
            def prep(s):
                with tc.tile_pool(name=f"prep{s}", bufs=1) as prep_p:
                    xh = [prep_p.tile([P, S], f32r, tag=f"xh{hc}", name=f"xh{hc}") for hc in range(8)]
                    for hc in range(8):
                        (nc.sync if hc % 2 == 0 else nc.gpsimd).dma_start(
                            out=xh[hc], in_=xh_d[s][hc * P:(hc + 1) * P, :])
                    cqk_sb = prep_p.tile([P, 8], f32, tag="cqk", name="cqk_sb")
                    nc.sync.dma_start(out=cqk_sb, in_=cqk_d[s][:])

                    # q/k projections in DR block order; q (blocks 0-3) first
                    for qk in range(2):
                        dst = (qdr if qk == 0 else kdr)[s]
                        for ht in range(2):
                            stg_t = [[None, None], [None, None]]  # [half][sc]
                            for half in range(2):
                                b = qk * 4 + ht * 2 + half
                                wqf = prep_p.tile([P, 8, P], f32r, tag="wqf", bufs=2, name="wqf")
                                (nc.sync if (ht + half) % 2 == 0 else nc.gpsimd).dma_start(
                                    out=wqf,
                                    in_=wqk_d[s][:, b * P:(b + 1) * P].rearrange("(c p) n -> p c n", p=P))
                                for sc in range(2):
                                    psq = pp.tile([P, 512], f32, tag="mm", name="psq")
                                    for kc in range(8):
                                        nc.tensor.matmul(
                                            psq,
                                            wqf[:, kc, :],
                                            xh[kc][:, sc * 512:(sc + 1) * 512],
                                            start=(kc == 0), stop=(kc == 7),
                                        )
                                    stg = prep_p.tile([P, 512], bf16, tag="stg", bufs=5, name="stg")
                                    nc.vector.tensor_scalar_add(stg, psq, cqk_sb[:, b:b + 1])
                                    stg_t[half][sc] = stg
                            # rope: out half i mixes staged lo/hi planes; the
                            # add splits per head-pair tile (2ht+g)
                            for i in range(2):
                                for sc in range(2):
                                    csl = slice(sc * 512, (sc + 1) * 512)
                                    tmp = small.tile([P, 512], bf16, tag="rtmp", bufs=3, name="rtmp")
                                    nc.vector.tensor_mul(tmp, stg_t[1 - i][sc], sindr[:, i, csl])
                                    qc = small.tile([P, 512], bf16, tag="rqc", bufs=3, name="rqc")
                                    nc.vector.tensor_mul(qc, stg_t[i][sc], cosdr[:, i, csl])
                                    nc.vector.tensor_add(dst[ht][:, i, csl], tmp, qc)

                    # V natural [s, n] + ones column, bf16
                    wvf = prep_p.tile([P, 8, NHL * D], f32r, tag="wvf", name="wvf")
                    nc.gpsimd.dma_start(out=wvf, in_=wv_d[s][:].rearrange("(c p) n -> p c n", p=P))
                    for st in range(8):
                        psv = pp.tile([P, 512], f32, tag="mm", name="psv")
                        for kc in range(8):
                            nc.tensor.matmul(
                                psv,
                                xh[kc][:, st * P:(st + 1) * P],
                                wvf[:, kc, :],
                                start=(kc == 0), stop=(kc == 7),
                            )
                        nc.vector.memset(v_sb[s][st][:, :, D:D + 1], 1.0)
                        nc.vector.tensor_copy(
                            out=v_sb[s][st][:, :, 0:D],
                            in_=psv.rearrange("p (h d) -> p h d", d=D),
                        )

            # ---------------- attention slot ----------------
            def slot(s, ht, hh, split):
                h = 4 * ht + hh
                pr = slice(32 * hh, 32 * hh + 32)
                tpos = (32 * hh, 0)
                row = (h % 2) * 64
                pair = h // 2

                def half_attn(acc, ts, start, stop):
                    for tst in range(8):
                        ps = scps.tile([P, S], f32, tag="sc", name="ps")
                        for sc in range(2):
                            csl = slice(sc * 512, (sc + 1) * 512)
                            nc.tensor.matmul(
                                ps[:, csl],
                                kdr[ts][ht][pr, :, tst * P:(tst + 1) * P],
                                qdr[s][ht][pr, :, csl],
                                perf_mode=DR,
                                tile_position=tpos,
                            )
                        es = es_pool.tile([P, S], bf16, tag="es", bufs=4, name="es")
                        nc.scalar.activation(out=es, in_=ps, func=AF.Exp, bias=zeroc, scale=SCALE)
                        for sc in range(2):
                            csl = slice(sc * 512, (sc + 1) * 512)
                            nc.tensor.matmul(
                                acc[:, csl],
                                v_sb[ts][tst][:, h, :],
                                es[:, csl],
                                start=(start and tst == 0), stop=(stop and tst == 7),
                            )

                def normalize(src):
                    rstg = small.tile([1, S], f32, tag="rstg", bufs=1, name="rstg")
                    nc.vector.reciprocal(out=rstg, in_=src[D:D + 1, :])
                    rbc = small.tile([D, S], f32, tag="rbc", bufs=1, name="rbc")
                    nc.gpsimd.partition_broadcast(rbc, rstg)
                    nc.vector.tensor_mul(attn[s][pair][row:row + D, :], src[0:D, :], rbc)

                if split:
                    acc = accps.tile([D + 1, S], f32, tag="acc", name="acc")
                    half_attn(acc, 0, True, True)
                    pa = pa_pool.tile([D + 1, S], f32, tag="pa", bufs=N_SPLIT, name="pa")
                    nc.vector.tensor_copy(out=pa, in_=acc)
                    acc = accps.tile([D + 1, S], f32, tag="acc", name="acc")
                    half_attn(acc, 1, True, True)
                    nsum = small.tile([D + 1, S], f32, tag="nsum", bufs=1, name="nsum")
                    nc.vector.tensor_add(nsum, acc, pa)
                    normalize(nsum)
                else:
                    acc = accps.tile([D + 1, S], f32, tag="acc", name="acc")
                    half_attn(acc, 0, True, False)
                    half_attn(acc, 1, False, True)
                    normalize(acc)

            # ---------------- tail: out-projection per stream ----------------
            def tail(s):
                with tc.tile_pool(name=f"wo{s}", bufs=1) as wop:
                    wo_t = [wop.tile([P, H], f32r, tag=f"wo{p}", name=f"wo{p}") for p in range(4)]
                    for p in range(4):
                        nc.sync.dma_start(out=wo_t[p], in_=wout_d[s][p * P:(p + 1) * P, :])
                    for st in range(8):
                        for oc in range(2):
                            pso = pp.tile([P, 512], f32, tag="mm", name="pso")
                            for p in range(4):
                                nc.tensor.matmul(
                                    pso,
                                    attn[s][p][:, st * P:(st + 1) * P],
                                    wo_t[p][:, oc * 512:(oc + 1) * 512],
                                    start=(p == 0), stop=(p == 3),
                                )
                            osb = small.tile([P, 512], f32, tag="osb", bufs=3, name="osb")
                            if s == 1 and (st + oc) % 2 == 0:
                                nc.scalar.copy(out=osb, in_=pso)
                            else:
                                nc.vector.tensor_copy(out=osb, in_=pso)
                            (nc.gpsimd if (st + oc) % 2 == 0 else nc.sync).dma_start(
                                out=out_d[s][st * P:(st + 1) * P, oc * 512:(oc + 1) * 512], in_=osb)

            # ---------------- emission ----------------
            prep(0)
            prep(1)
            idx = 0
            for s in range(2):
                for ht in range(2):
                    for hh in range(4):
                        slot(s, ht, hh, split=(idx < N_SPLIT))
                        idx += 1
                tail(s)

    if do_compile:
        nc.compile()
    return nc


def _host_prep(x_a, x_b, Wqkv_a, Wqkv_b, Wout_a, Wout_b,
               gamma_a, beta_a, gamma_b, beta_b, height, width):
    """Build the 8 per-core input maps."""
    import ml_dtypes
    cos, sin = _rope_tables(height, width)      # [S, 64]

    # DR rope tables [128, 2, S]: partition p = 32*hh + dl (repeats over hh)
    dl = np.arange(32)
    cos_dr = np.empty((P, 2, S), np.float32)
    sin_dr = np.empty((P, 2, S), np.float32)
    for hh in range(4):
        rows = 32 * hh + dl
        cos_dr[rows, 0, :] = cos[:, dl].T          # cos[s, dl]
        cos_dr[rows, 1, :] = cos[:, 32 + dl].T
        sin_dr[rows, 0, :] = -sin[:, dl].T         # pre-negated for i=0
        sin_dr[rows, 1, :] = sin[:, 32 + dl].T
    cos_dr = np.ascontiguousarray(cos_dr.reshape(P, 2 * S).astype(ml_dtypes.bfloat16))
    sin_dr = np.ascontiguousarray(sin_dr.reshape(P, 2 * S).astype(ml_dtypes.bfloat16))

    # host LayerNorm fold: xhat = r*(x-mu), shipped transposed [H, S]
    def xhat(x):
        x = x.astype(np.float32)
        mu = x.mean(axis=-1, keepdims=True)
        var = ((x - mu) ** 2).mean(axis=-1, keepdims=True)
        r = 1.0 / np.sqrt(var + LN_EPS)
        return ((x - mu) * r).astype(np.float32)

    streams = []
    vshifts = []
    for (W, Wo, g, b) in ((Wqkv_a, Wout_a, gamma_a, beta_a), (Wqkv_b, Wout_b, gamma_b, beta_b)):
        Wg = (W * g[:, None]).astype(np.float32)       # gamma-folded
        cfull = (b.astype(np.float64) @ W.astype(np.float64)).astype(np.float32)  # beta@W [3H]
        W4 = Wg.reshape(H, 3, NH, D)
        c4 = cfull.reshape(3, NH, D)
        per_hg = []
        for hg in range(2):
            h0 = hg * NHL
            # DR column order: blocks (qk, ht, half) of 128 cols = (hh, dl)
            cols = []
            ccols = []
            for qk in range(2):
                for ht in range(2):
                    for half in range(2):
                        for hh in range(4):
                            head = h0 + 4 * ht + hh
                            dsl = slice(32 * half, 32 * half + 32)
                            cols.append(W4[:, qk, head, dsl])      # [H, 32]
                            ccols.append(c4[qk, head, dsl])        # [32]
            wqk = np.ascontiguousarray(np.concatenate(cols, axis=1))       # [H, 1024]
            # beta@W per qk column as per-partition scalars [128, 8 blocks]
            cqk_blk = np.ascontiguousarray(
                np.concatenate(ccols).reshape(8, P).T.astype(np.float32))   # [128, 8]
            wv = np.ascontiguousarray(W4[:, 2, h0:h0 + NHL, :].reshape(H, NHL * D))
            wout = np.ascontiguousarray(Wo.reshape(NH, D, H)[h0:h0 + NHL].reshape(NHL * D, H).astype(np.float32))
            per_hg.append(dict(wqk=wqk, wv=wv, cqk=cqk_blk, wout=wout))
        # exact host-side V correction: beta@Wv shifts attn uniformly
        # (softmax weights sum to 1), so it lands as a constant row on out
        vshift = (cfull[2 * H:3 * H].astype(np.float64) @ Wo.astype(np.float64)).astype(np.float32)
        streams.append(per_hg)
        vshifts.append(vshift)

    in_maps = []
    B = x_a.shape[0]
    xh_a = [np.ascontiguousarray(xhat(x_a[b_i]).T) for b_i in range(B)]
    xh_b = [np.ascontiguousarray(xhat(x_b[b_i]).T) for b_i in range(B)]
    for c in range(N_CORES):
        b_i, hg = (c // 2) % B, c % 2
        m = {
            "xh_s0": xh_a[b_i],
            "xh_s1": xh_b[b_i],
            "cosdr": cos_dr, "sindr": sin_dr,
        }
        for s in range(2):
            blk = streams[s][hg]
            m[f"wqk_s{s}"] = blk["wqk"]
            m[f"wv_s{s}"] = blk["wv"]
            m[f"cqk_s{s}"] = blk["cqk"]
            m[f"wout_s{s}"] = blk["wout"]
        in_maps.append(m)
    return in_maps, vshifts


def kernel(x_a, x_b, Wqkv_a, Wqkv_b, Wout_a, Wout_b,
           gamma_a, beta_a, gamma_b, beta_b, height, width):
    from concourse.bass_utils import run_bass_kernel_spmd

    x_a = np.asarray(x_a, dtype=np.float32)
    x_b = np.asarray(x_b, dtype=np.float32)
    B = x_a.shape[0]
    in_maps, vshifts = _host_prep(x_a, x_b,
                         np.asarray(Wqkv_a, np.float32), np.asarray(Wqkv_b, np.float32),
                         np.asarray(Wout_a, np.float32), np.asarray(Wout_b, np.float32),
                         np.asarray(gamma_a, np.float32), np.asarray(beta_a, np.float32),
                         np.asarray(gamma_b, np.float32), np.asarray(beta_b, np.float32),
                         height, width)
    nc = _get_program()
    res = run_bass_kernel_spmd(nc, in_maps, list(range(N_CORES))).results
    out_a = np.empty((B, S, H), np.float32)
    out_b = np.empty((B, S, H), np.float32)
    for b_i in range(B):
        out_a[b_i] = res[2 * b_i]["out_s0"] + res[2 * b_i + 1]["out_s0"] + vshifts[0]
        out_b[b_i] = res[2 * b_i]["out_s1"] + res[2 * b_i + 1]["out_s1"] + vshifts[1]
    return out_a, out_b


def _get_program():
    global _PROGRAM
    if _PROGRAM is None:
        _PROGRAM = _build_program()
    return _PROGRAM


# revision 14
# speedup vs baseline: 1.5989x; 1.0546x over previous
"""DualStreamEncoderAttention Trainium2 kernel (v2).

Sharding: 8 cores = 4 samples x 2 head-groups (8 heads each). Each core
computes, for its sample, both streams' QKV(+RoPE) for its 8 heads,
cross-stream attention (KV concat is per-sample, head sharding is clean),
and a partial out-projection over its heads' rows of Wout. The host sums
the two partial projections per sample. No collectives; pure SPMD.

Speed strategy (S=1024, H=1024, D=64, 8 local heads):
  - LayerNorm is folded on the host: the kernel receives
    xhat = r*(x-mu) pre-transposed to [H, S] (r,mu are pure input
    functions), plus gamma folded into W and a rank-1 beta@W correction
    row added via a K=1 matmul into the same PSUM accumulation.
  - Q,K are produced in a DoubleRow layout: per (stream, ht) a tile
    [128 = 4 heads x 32 dlow, 2 d-halves, S] in fp8e4. QK^T scores run
    as fp8 DoubleRow matmuls (0.5 cycles/col, contraction 2x32=64 on 32
    partitions) - 2x the bf16 PE rate.
  - RoPE's rotate-half partner (d <-> d+32) lives on the same partition
    in the other d-half plane, so the rotation is plain elementwise math
    between the lo/hi projection halves - no partition-shuffle DMAs.
    Sin tables are pre-negated on the host; the final add emits fp8.
  - exp on the Scalar engine from PSUM ([128,1024] tiles, scale=1/8
    folded in), output bf16; softmax denominator via a ones-column in V
    (PV accumulator row 64). PV is bf16.
  - Emission order lets the Tile scheduler overlap everything: stream-a
    prep -> attention over stream-a keys starts (~20us in) while
    stream-b prep runs on PE/DVE. Early slots bridge the a->b key gap by
    draining partial PV accumulators to SBUF. The Activation engine
    (256 x [128,1024] exp = the hard floor) stays ~saturated; all PSUM
    evictions are on DVE/Pool.
"""

import sys

for _p in ("/opt/trn_rl_repo", "/root/.axon_site/_ro/trn_rl_repo"):
    if _p not in sys.path:
        sys.path.insert(0, _p)

import numpy as np

S = 1024
H = 1024
NH = 16
D = 64
NHL = 8          # heads per core
P = 128
N_CORES = 8
LN_EPS = 1e-5
ROPE_BASE = 10000.0
SCALE = float(D) ** -0.5

# leading attention slots split PV accumulation into stream-a / stream-b
# phases (bridges the stream-b prep gap)
N_SPLIT = 6

_PROGRAM = None


def _rope_tables(height, width, head_dim=D):
    """Mirror of reference.rope_2d_tables in numpy float32."""
    height = int(height)
    width = int(width)
    dim_x = head_dim // 2
    dim_y = head_dim - dim_x
    inv_fx = 1.0 / (ROPE_BASE ** (np.arange(0, dim_x, 2, dtype=np.float32) / np.float32(dim_x)))
    inv_fy = 1.0 / (ROPE_BASE ** (np.arange(0, dim_y, 2, dtype=np.float32) / np.float32(dim_y)))
    fx = np.arange(width, dtype=np.float32)[:, None] * inv_fx[None, :]
    fy = np.arange(height, dtype=np.float32)[:, None] * inv_fy[None, :]
    fx = np.concatenate([fx, fx], axis=-1)  # [W, dim_x]
    fy = np.concatenate([fy, fy], axis=-1)  # [H, dim_y]
    cos = np.concatenate([
        np.broadcast_to(np.cos(fx)[None, :, :], (height, width, dim_x)),
        np.broadcast_to(np.cos(fy)[:, None, :], (height, width, dim_y)),
    ], axis=-1).reshape(height * width, head_dim).astype(np.float32)
    sin = np.concatenate([
        np.broadcast_to(np.sin(fx)[None, :, :], (height, width, dim_x)),
        np.broadcast_to(np.sin(fy)[:, None, :], (height, width, dim_y)),
    ], axis=-1).reshape(height * width, head_dim).astype(np.float32)
    return cos, sin


def _build_program(do_compile=True):
    import concourse.mybir as mybir
    import concourse.tile as tile
    from concourse import bacc

    f32 = mybir.dt.float32
    f32r = mybir.dt.float32r
    bf16 = mybir.dt.bfloat16
    fp8 = mybir.dt.float8e4
    DR = mybir.MatmulPerfMode.DoubleRow
    AF = mybir.ActivationFunctionType

    nc = bacc.Bacc("TRN2")

    # ---- DRAM parameters (per-core tensors; same program on all cores) ----
    # xhat: r*(x-mu), pre-transposed [H, S]
    xh_d = [nc.dram_tensor(f"xh_s{s}", [H, S], f32r, kind="ExternalInput") for s in range(2)]
    # qk weights (gamma-folded), columns in DR order: 8 blocks of 128 = (q|k, ht, half)
    wqk_d = [nc.dram_tensor(f"wqk_s{s}", [H, 2 * NHL * D], f32r, kind="ExternalInput") for s in range(2)]
    wv_d = [nc.dram_tensor(f"wv_s{s}", [H, NHL * D], f32r, kind="ExternalInput") for s in range(2)]
    cqk_d = [nc.dram_tensor(f"cqk_s{s}", [P, 8], f32, kind="ExternalInput") for s in range(2)]
    wout_d = [nc.dram_tensor(f"wout_s{s}", [NHL * D, H], f32r, kind="ExternalInput") for s in range(2)]
    cos_d = nc.dram_tensor("cosdr", [P, 2 * S], bf16, kind="ExternalInput")
    sin_d = nc.dram_tensor("sindr", [P, 2 * S], bf16, kind="ExternalInput")  # pre-negated i=0 half
    out_d = [nc.dram_tensor(f"out_s{s}", [S, H], f32, kind="ExternalOutput") for s in range(2)]

    with tile.TileContext(nc) as tc:
        with (
            tc.tile_pool(name="consts", bufs=1) as consts,
            tc.tile_pool(name="persist", bufs=1) as persist,
            tc.tile_pool(name="attx", bufs=1) as attx,
            tc.tile_pool(name="small", bufs=2) as small,
            tc.tile_pool(name="pp", bufs=2, space="PSUM") as pp,
            tc.tile_pool(name="scps", bufs=2, space="PSUM") as scps,
            tc.tile_pool(name="accps", bufs=1, space="PSUM") as accps,
        ):
            cosdr = consts.tile([P, 2, S], bf16, tag="cosdr")
            nc.sync.dma_start(out=cosdr, in_=cos_d[:].rearrange("p (i s) -> p i s", i=2))
            sindr = consts.tile([P, 2, S], bf16, tag="sindr")
            nc.sync.dma_start(out=sindr, in_=sin_d[:].rearrange("p (i s) -> p i s", i=2))
            zeroc = consts.tile([P, 1], f32, tag="zeroc")
            nc.vector.memset(zeroc, 0.0)

            # persistent per-stream state; q/k as 4-head DR tiles
            # [128 = 4 heads x 32 dlow, 2 d-halves, S] fp8
            qdr = [[persist.tile([P, 2, S], fp8, tag=f"qdr{s}_{ht}", name=f"qdr{s}_{ht}")
                    for ht in range(2)] for s in range(2)]
            kdr = [[persist.tile([P, 2, S], fp8, tag=f"kdr{s}_{ht}", name=f"kdr{s}_{ht}")
                    for ht in range(2)] for s in range(2)]
            v_sb = [[persist.tile([P, NHL, D + 1], bf16, tag=f"v{s}_{st}", name=f"v{s}_{st}")
                     for st in range(8)] for s in range(2)]
            attn = [[persist.tile([P, S], f32r, tag=f"attn{s}_{p}", name=f"attn{s}_{p}")
                     for p in range(4)] for s in range(2)]

            es_pool = attx
            pa_pool = attx

            # ---------------- prep per stream ----------------
            def prep(s):
                with tc.tile_pool(name=f"prep{s}", bufs=1) as prep_p:
                    xh = [prep_p.tile([P, S], f32r, tag=f"xh{hc}", name=f"xh{hc}") for hc in range(8)]
                    for hc in range(8):
                        (nc.sync if hc % 2 == 0 else nc.gpsimd).dma_start(
                            out=xh[hc], in_=xh_d[s][hc * P:(hc + 1) * P, :])
                    cqk_sb = prep_p.tile([P, 8], f32, tag="cqk", name="cqk_sb")
                    nc.sync.dma_start(out=cqk_sb, in_=cqk_d[s][:])

                    # q/k/V projection fills, ordered so ht0 attention and
                    # PV can start while later fills stream: q-ht0, k-ht0,
                    # V, q-ht1, k-ht1
                    def qk_fills(qk, ht):
                        dst = (qdr if qk == 0 else kdr)[s]
                        stg_t = [[None, None], [None, None]]  # [half][sc]
                        for half in range(2):
                            b = qk * 4 + ht * 2 + half
                            wqf = prep_p.tile([P, 8, P], f32r, tag="wqf", bufs=2, name="wqf")
                            (nc.sync if (ht + half) % 2 == 0 else nc.gpsimd).dma_start(
                                out=wqf,
                                in_=wqk_d[s][:, b * P:(b + 1) * P].rearrange("(c p) n -> p c n", p=P))
                            for sc in range(2):
                                psq = pp.tile([P, 512], f32, tag="mm", name="psq")
                                for kc in range(8):
                                    nc.tensor.matmul(
                                        psq,
                                        wqf[:, kc, :],
                                        xh[kc][:, sc * 512:(sc + 1) * 512],
                                        start=(kc == 0), stop=(kc == 7),
                                    )
                                stg = prep_p.tile([P, 512], bf16, tag="stg", bufs=5, name="stg")
                                nc.vector.tensor_scalar_add(stg, psq, cqk_sb[:, b:b + 1])
                                stg_t[half][sc] = stg
                        # rope: out half i mixes the staged lo/hi planes
                        for i in range(2):
                            for sc in range(2):
                                csl = slice(sc * 512, (sc + 1) * 512)
                                tmp = small.tile([P, 512], bf16, tag="rtmp", bufs=3, name="rtmp")
                                nc.vector.tensor_mul(tmp, stg_t[1 - i][sc], sindr[:, i, csl])
                                qc = small.tile([P, 512], bf16, tag="rqc", bufs=3, name="rqc")
                                nc.vector.tensor_mul(qc, stg_t[i][sc], cosdr[:, i, csl])
                                nc.vector.tensor_add(dst[ht][:, i, csl], tmp, qc)

                    def v_fills():
                        wvf = prep_p.tile([P, 8, NHL * D], f32r, tag="wvf", name="wvf")
                        nc.gpsimd.dma_start(out=wvf, in_=wv_d[s][:].rearrange("(c p) n -> p c n", p=P))
                        for st in range(8):
                            psv = pp.tile([P, 512], f32, tag="mm", name="psv")
                            for kc in range(8):
                                nc.tensor.matmul(
                                    psv,
                                    xh[kc][:, st * P:(st + 1) * P],
                                    wvf[:, kc, :],
                                    start=(kc == 0), stop=(kc == 7),
                                )
                            nc.vector.memset(v_sb[s][st][:, :, D:D + 1], 1.0)
                            nc.vector.tensor_copy(
                                out=v_sb[s][st][:, :, 0:D],
                                in_=psv.rearrange("p (h d) -> p h d", d=D),
                            )

                    qk_fills(0, 0)
                    qk_fills(1, 0)
                    v_fills()
                    qk_fills(0, 1)
                    qk_fills(1, 1)

            # ---------------- attention slots (phase-major) ----------------
            pa_t = {}

            def half_attn(acc, s, ht, hh, ts):
                h = 4 * ht + hh
                pr = slice(32 * hh, 32 * hh + 32)
                tpos = (32 * hh, 0)
                for tst in range(8):
                    ps = scps.tile([P, S], f32, tag="sc", name="ps")
                    for sc in range(2):
                        csl = slice(sc * 512, (sc + 1) * 512)
                        nc.tensor.matmul(
                            ps[:, csl],
                            kdr[ts][ht][pr, :, tst * P:(tst + 1) * P],
                            qdr[s][ht][pr, :, csl],
                            perf_mode=DR,
                            tile_position=tpos,
                        )
                    es = es_pool.tile([P, S], bf16, tag="es", bufs=6, name="es")
                    nc.scalar.activation(out=es, in_=ps, func=AF.Exp, bias=zeroc, scale=SCALE)
                    for sc in range(2):
                        csl = slice(sc * 512, (sc + 1) * 512)
                        nc.tensor.matmul(
                            acc[:, csl],
                            v_sb[ts][tst][:, h, :],
                            es[:, csl],
                            start=(tst == 0), stop=(tst == 7),
                        )

            def slot_a(s, ht, hh):
                acc = accps.tile([D + 1, S], f32, tag="acc", name="acc")
                half_attn(acc, s, ht, hh, 0)
                pa = pa_pool.tile([D + 1, S], bf16, tag="pa", bufs=16, name="pa")
                nc.vector.tensor_copy(out=pa, in_=acc)
                pa_t[(s, ht, hh)] = pa

            def slot_b(s, ht, hh):
                h = 4 * ht + hh
                acc = accps.tile([D + 1, S], f32, tag="acc", name="acc")
                half_attn(acc, s, ht, hh, 1)
                nsum = small.tile([D + 1, S], f32, tag="nsum", bufs=2, name="nsum")
                nc.vector.tensor_add(nsum, acc, pa_t[(s, ht, hh)])
                rstg = small.tile([1, S], f32, tag="rstg", bufs=1, name="rstg")
                nc.vector.reciprocal(out=rstg, in_=nsum[D:D + 1, :])
                rbc = small.tile([D, S], f32, tag="rbc", bufs=1, name="rbc")
                nc.gpsimd.partition_broadcast(rbc, rstg)
                nc.vector.tensor_mul(attn[s][h // 2][(h % 2) * 64:(h % 2) * 64 + D, :],
                                     nsum[0:D, :], rbc)

            # ---------------- tail: out-projection per stream ----------------
            def tail(s):
                with tc.tile_pool(name=f"wo{s}", bufs=1) as wop:
                    wo_t = [wop.tile([P, H], f32r, tag=f"wo{p}", name=f"wo{p}") for p in range(4)]
                    for p in range(4):
                        nc.sync.dma_start(out=wo_t[p], in_=wout_d[s][p * P:(p + 1) * P, :])
                    for st in range(8):
                        for oc in range(2):
                            pso = pp.tile([P, 512], f32, tag="mm", name="pso")
                            for p in range(4):
                                nc.tensor.matmul(
                                    pso,
                                    attn[s][p][:, st * P:(st + 1) * P],
                                    wo_t[p][:, oc * 512:(oc + 1) * 512],
                                    start=(p == 0), stop=(p == 3),
                                )
                            osb = small.tile([P, 512], f32, tag="osb", bufs=3, name="osb")
                            if s == 1 and (st + oc) % 2 == 0:
                                nc.scalar.copy(out=osb, in_=pso)
                            else:
                                nc.vector.tensor_copy(out=osb, in_=pso)
                            (nc.gpsimd if (st + oc) % 2 == 0 else nc.sync).dma_start(
                                out=out_d[s][st * P:(st + 1) * P, oc * 512:(oc + 1) * 512], in_=osb)

            # ---------------- emission ----------------
            prep(0)
            prep(1)
            order = [(s, ht, hh) for s in range(2) for ht in range(2) for hh in range(4)]
            for (s, ht, hh) in order:
                slot_a(s, ht, hh)
            for (s, ht, hh) in order:
                slot_b(s, ht, hh)
                if (s, ht, hh) == order[7]:
                    tail(0)
            tail(1)

    if do_compile:
        nc.compile()
    return nc


def _host_prep(x_a, x_b, Wqkv_a, Wqkv_b, Wout_a, Wout_b,
               gamma_a, beta_a, gamma_b, beta_b, height, width):
    """Build the 8 per-core input maps."""
    import ml_dtypes
    cos, sin = _rope_tables(height, width)      # [S, 64]

    # DR rope tables [128, 2, S]: partition p = 32*hh + dl (repeats over hh)
    dl = np.arange(32)
    cos_dr = np.empty((P, 2, S), np.float32)
    sin_dr = np.empty((P, 2, S), np.float32)
    for hh in range(4):
        rows = 32 * hh + dl
        cos_dr[rows, 0, :] = cos[:, dl].T          # cos[s, dl]
        cos_dr[rows, 1, :] = cos[:, 32 + dl].T
        sin_dr[rows, 0, :] = -sin[:, dl].T         # pre-negated for i=0
        sin_dr[rows, 1, :] = sin[:, 32 + dl].T
    cos_dr = np.ascontiguousarray(cos_dr.reshape(P, 2 * S).astype(ml_dtypes.bfloat16))
    sin_dr = np.ascontiguousarray(sin_dr.reshape(P, 2 * S).astype(ml_dtypes.bfloat16))

    # host LayerNorm fold: xhat = r*(x-mu), shipped transposed [H, S]
    def xhat(x):
        x = x.astype(np.float32)
        mu = x.mean(axis=-1, keepdims=True)
        var = ((x - mu) ** 2).mean(axis=-1, keepdims=True)
        r = 1.0 / np.sqrt(var + LN_EPS)
        return ((x - mu) * r).astype(np.float32)

    streams = []
    vshifts = []
    for (W, Wo, g, b) in ((Wqkv_a, Wout_a, gamma_a, beta_a), (Wqkv_b, Wout_b, gamma_b, beta_b)):
        Wg = (W * g[:, None]).astype(np.float32)       # gamma-folded
        cfull = (b.astype(np.float64) @ W.astype(np.float64)).astype(np.float32)  # beta@W [3H]
        W4 = Wg.reshape(H, 3, NH, D)
        c4 = cfull.reshape(3, NH, D)
        per_hg = []
        for hg in range(2):
            h0 = hg * NHL
            # DR column order: blocks (qk, ht, half) of 128 cols = (hh, dl)
            cols = []
            ccols = []
            for qk in range(2):
                for ht in range(2):
                    for half in range(2):
                        for hh in range(4):
                            head = h0 + 4 * ht + hh
                            dsl = slice(32 * half, 32 * half + 32)
                            cols.append(W4[:, qk, head, dsl])      # [H, 32]
                            ccols.append(c4[qk, head, dsl])        # [32]
            wqk = np.ascontiguousarray(np.concatenate(cols, axis=1))       # [H, 1024]
            # beta@W per qk column as per-partition scalars [128, 8 blocks]
            cqk_blk = np.ascontiguousarray(
                np.concatenate(ccols).reshape(8, P).T.astype(np.float32))   # [128, 8]
            wv = np.ascontiguousarray(W4[:, 2, h0:h0 + NHL, :].reshape(H, NHL * D))
            wout = np.ascontiguousarray(Wo.reshape(NH, D, H)[h0:h0 + NHL].reshape(NHL * D, H).astype(np.float32))
            per_hg.append(dict(wqk=wqk, wv=wv, cqk=cqk_blk, wout=wout))
        # exact host-side V correction: beta@Wv shifts attn uniformly
        # (softmax weights sum to 1), so it lands as a constant row on out
        vshift = (cfull[2 * H:3 * H].astype(np.float64) @ Wo.astype(np.float64)).astype(np.float32)
        streams.append(per_hg)
        vshifts.append(vshift)

    in_maps = []
    B = x_a.shape[0]
    xh_a = [np.ascontiguousarray(xhat(x_a[b_i]).T) for b_i in range(B)]
    xh_b = [np.ascontiguousarray(xhat(x_b[b_i]).T) for b_i in range(B)]
    for c in range(N_CORES):
        b_i, hg = (c // 2) % B, c % 2
        m = {
            "xh_s0": xh_a[b_i],
            "xh_s1": xh_b[b_i],
            "cosdr": cos_dr, "sindr": sin_dr,
        }
        for s in range(2):
            blk = streams[s][hg]
            m[f"wqk_s{s}"] = blk["wqk"]
            m[f"wv_s{s}"] = blk["wv"]
            m[f"cqk_s{s}"] = blk["cqk"]
            m[f"wout_s{s}"] = blk["wout"]
        in_maps.append(m)
    return in_maps, vshifts


def kernel(x_a, x_b, Wqkv_a, Wqkv_b, Wout_a, Wout_b,
           gamma_a, beta_a, gamma_b, beta_b, height, width):
    from concourse.bass_utils import run_bass_kernel_spmd

    x_a = np.asarray(x_a, dtype=np.float32)
    x_b = np.asarray(x_b, dtype=np.float32)
    B = x_a.shape[0]
    in_maps, vshifts = _host_prep(x_a, x_b,
                         np.asarray(Wqkv_a, np.float32), np.asarray(Wqkv_b, np.float32),
                         np.asarray(Wout_a, np.float32), np.asarray(Wout_b, np.float32),
                         np.asarray(gamma_a, np.float32), np.asarray(beta_a, np.float32),
                         np.asarray(gamma_b, np.float32), np.asarray(beta_b, np.float32),
                         height, width)
    nc = _get_program()
    res = run_bass_kernel_spmd(nc, in_maps, list(range(N_CORES))).results
    out_a = np.empty((B, S, H), np.float32)
    out_b = np.empty((B, S, H), np.float32)
    for b_i in range(B):
        out_a[b_i] = res[2 * b_i]["out_s0"] + res[2 * b_i + 1]["out_s0"] + vshifts[0]
        out_b[b_i] = res[2 * b_i]["out_s1"] + res[2 * b_i + 1]["out_s1"] + vshifts[1]
    return out_a, out_b


def _get_program():
    global _PROGRAM
    if _PROGRAM is None:
        _PROGRAM = _build_program()
    return _PROGRAM


# revision 15
# speedup vs baseline: 1.7542x; 1.0971x over previous
"""DualStreamEncoderAttention Trainium2 kernel (v2).

Sharding: 8 cores = 4 samples x 2 head-groups (8 heads each). Each core
computes, for its sample, both streams' QKV(+RoPE) for its 8 heads,
cross-stream attention (KV concat is per-sample, head sharding is clean),
and a partial out-projection over its heads' rows of Wout. The host sums
the two partial projections per sample. No collectives; pure SPMD.

Speed strategy (S=1024, H=1024, D=64, 8 local heads):
  - LayerNorm is folded on the host: the kernel receives
    xhat = r*(x-mu) pre-transposed to [H, S] (r,mu are pure input
    functions), plus gamma folded into W and a rank-1 beta@W correction
    row added via a K=1 matmul into the same PSUM accumulation.
  - Q,K are produced in a DoubleRow layout: per (stream, ht) a tile
    [128 = 4 heads x 32 dlow, 2 d-halves, S] in fp8e4. QK^T scores run
    as fp8 DoubleRow matmuls (0.5 cycles/col, contraction 2x32=64 on 32
    partitions) - 2x the bf16 PE rate.
  - RoPE's rotate-half partner (d <-> d+32) lives on the same partition
    in the other d-half plane, so the rotation is plain elementwise math
    between the lo/hi projection halves - no partition-shuffle DMAs.
    Sin tables are pre-negated on the host; the final add emits fp8.
  - exp on the Scalar engine from PSUM ([128,1024] tiles, scale=1/8
    folded in), output bf16; softmax denominator via a ones-column in V
    (PV accumulator row 64). PV is bf16.
  - Emission order lets the Tile scheduler overlap everything: stream-a
    prep -> attention over stream-a keys starts (~20us in) while
    stream-b prep runs on PE/DVE. Early slots bridge the a->b key gap by
    draining partial PV accumulators to SBUF. The Activation engine
    (256 x [128,1024] exp = the hard floor) stays ~saturated; all PSUM
    evictions are on DVE/Pool.
"""

import sys

for _p in ("/opt/trn_rl_repo", "/root/.axon_site/_ro/trn_rl_repo"):
    if _p not in sys.path:
        sys.path.insert(0, _p)

import numpy as np

S = 1024
H = 1024
NH = 16
D = 64
NHL = 8          # heads per core
P = 128
N_CORES = 8
LN_EPS = 1e-5
ROPE_BASE = 10000.0
SCALE = float(D) ** -0.5

# leading attention slots split PV accumulation into stream-a / stream-b
# phases (bridges the stream-b prep gap)
N_SPLIT = 6

_PROGRAM = None


def _rope_tables(height, width, head_dim=D):
    """Mirror of reference.rope_2d_tables in numpy float32."""
    height = int(height)
    width = int(width)
    dim_x = head_dim // 2
    dim_y = head_dim - dim_x
    inv_fx = 1.0 / (ROPE_BASE ** (np.arange(0, dim_x, 2, dtype=np.float32) / np.float32(dim_x)))
    inv_fy = 1.0 / (ROPE_BASE ** (np.arange(0, dim_y, 2, dtype=np.float32) / np.float32(dim_y)))
    fx = np.arange(width, dtype=np.float32)[:, None] * inv_fx[None, :]
    fy = np.arange(height, dtype=np.float32)[:, None] * inv_fy[None, :]
    fx = np.concatenate([fx, fx], axis=-1)  # [W, dim_x]
    fy = np.concatenate([fy, fy], axis=-1)  # [H, dim_y]
    cos = np.concatenate([
        np.broadcast_to(np.cos(fx)[None, :, :], (height, width, dim_x)),
        np.broadcast_to(np.cos(fy)[:, None, :], (height, width, dim_y)),
    ], axis=-1).reshape(height * width, head_dim).astype(np.float32)
    sin = np.concatenate([
        np.broadcast_to(np.sin(fx)[None, :, :], (height, width, dim_x)),
        np.broadcast_to(np.sin(fy)[:, None, :], (height, width, dim_y)),
    ], axis=-1).reshape(height * width, head_dim).astype(np.float32)
    return cos, sin


def _build_program(do_compile=True):
    import concourse.mybir as mybir
    import concourse.tile as tile
    from concourse import bacc

    f32 = mybir.dt.float32
    f32r = mybir.dt.float32r
    bf16 = mybir.dt.bfloat16
    fp8 = mybir.dt.float8e4
    DR = mybir.MatmulPerfMode.DoubleRow
    AF = mybir.ActivationFunctionType

    nc = bacc.Bacc("TRN2")

    # ---- DRAM parameters (per-core tensors; same program on all cores) ----
    # xhat: r*(x-mu), pre-transposed [H, S]
    xh_d = [nc.dram_tensor(f"xh_s{s}", [H, S], f32r, kind="ExternalInput") for s in range(2)]
    # qk weights (gamma-folded), columns in DR order: 8 blocks of 128 = (q|k, ht, half)
    wqk_d = [nc.dram_tensor(f"wqk_s{s}", [H, 2 * NHL * D], f32r, kind="ExternalInput") for s in range(2)]
    wv_d = [nc.dram_tensor(f"wv_s{s}", [H, NHL * D], f32r, kind="ExternalInput") for s in range(2)]
    cqk_d = [nc.dram_tensor(f"cqk_s{s}", [P, 8], f32, kind="ExternalInput") for s in range(2)]
    wout_d = [nc.dram_tensor(f"wout_s{s}", [NHL * D, H], f32r, kind="ExternalInput") for s in range(2)]
    cos_d = nc.dram_tensor("cosdr", [P, 2 * S], bf16, kind="ExternalInput")
    sin_d = nc.dram_tensor("sindr", [P, 2 * S], bf16, kind="ExternalInput")  # pre-negated i=0 half
    out_d = [nc.dram_tensor(f"out_s{s}", [S, H], f32, kind="ExternalOutput") for s in range(2)]

    with tile.TileContext(nc) as tc:
        with (
            tc.tile_pool(name="consts", bufs=1) as consts,
            tc.tile_pool(name="persist", bufs=1) as persist,
            tc.tile_pool(name="attx", bufs=1) as attx,
            tc.tile_pool(name="small", bufs=2) as small,
            tc.tile_pool(name="pp", bufs=2, space="PSUM") as pp,
            tc.tile_pool(name="scps", bufs=2, space="PSUM") as scps,
            tc.tile_pool(name="accps", bufs=1, space="PSUM") as accps,
        ):
            cosdr = consts.tile([P, 2, S], bf16, tag="cosdr")
            nc.sync.dma_start(out=cosdr, in_=cos_d[:].rearrange("p (i s) -> p i s", i=2))
            sindr = consts.tile([P, 2, S], bf16, tag="sindr")
            nc.sync.dma_start(out=sindr, in_=sin_d[:].rearrange("p (i s) -> p i s", i=2))
            zeroc = consts.tile([P, 1], f32, tag="zeroc")
            nc.vector.memset(zeroc, 0.0)

            # persistent per-stream state; q/k as 4-head DR tiles
            # [128 = 4 heads x 32 dlow, 2 d-halves, S] fp8
            qdr = [[persist.tile([P, 2, S], fp8, tag=f"qdr{s}_{ht}", name=f"qdr{s}_{ht}")
                    for ht in range(2)] for s in range(2)]
            kdr = [[persist.tile([P, 2, S], fp8, tag=f"kdr{s}_{ht}", name=f"kdr{s}_{ht}")
                    for ht in range(2)] for s in range(2)]
            v_sb = [[persist.tile([P, NHL, D + 1], bf16, tag=f"v{s}_{st}", name=f"v{s}_{st}")
                     for st in range(8)] for s in range(2)]
            attn = [[persist.tile([P, S], f32r, tag=f"attn{s}_{p}", name=f"attn{s}_{p}")
                     for p in range(4)] for s in range(2)]

            es_pool = attx
            pa_pool = attx

            # ---------------- prep helpers ----------------
            def load_stream(s, prep_p):
                xh = [prep_p.tile([P, S], f32r, tag=f"xh{hc}", name=f"xh{hc}") for hc in range(8)]
                for hc in range(8):
                    (nc.sync if hc % 2 == 0 else nc.gpsimd).dma_start(
                        out=xh[hc], in_=xh_d[s][hc * P:(hc + 1) * P, :])
                cqk_sb = prep_p.tile([P, 8], f32, tag="cqk", bufs=2, name="cqk_sb")
                nc.sync.dma_start(out=cqk_sb, in_=cqk_d[s][:])
                return xh, cqk_sb

            def qk_fills(s, xh, cqk_sb, qk, ht, prep_p):
                dst = (qdr if qk == 0 else kdr)[s]
                stg_t = [[None, None], [None, None]]  # [half][sc]
                for half in range(2):
                    b = qk * 4 + ht * 2 + half
                    wqf = prep_p.tile([P, 8, P], f32r, tag="wqf", bufs=2, name="wqf")
                    (nc.sync if (ht + half) % 2 == 0 else nc.gpsimd).dma_start(
                        out=wqf,
                        in_=wqk_d[s][:, b * P:(b + 1) * P].rearrange("(c p) n -> p c n", p=P))
                    for sc in range(2):
                        psq = pp.tile([P, 512], f32, tag="mm", name="psq")
                        for kc in range(8):
                            nc.tensor.matmul(
                                psq,
                                wqf[:, kc, :],
                                xh[kc][:, sc * 512:(sc + 1) * 512],
                                start=(kc == 0), stop=(kc == 7),
                            )
                        stg = prep_p.tile([P, 512], bf16, tag="stg", bufs=5, name="stg")
                        nc.vector.tensor_scalar_add(stg, psq, cqk_sb[:, b:b + 1])
                        stg_t[half][sc] = stg
                # rope: out half i mixes the staged lo/hi planes
                for i in range(2):
                    for sc in range(2):
                        csl = slice(sc * 512, (sc + 1) * 512)
                        tmp = small.tile([P, 512], bf16, tag="rtmp", bufs=3, name="rtmp")
                        nc.vector.tensor_mul(tmp, stg_t[1 - i][sc], sindr[:, i, csl])
                        qc = small.tile([P, 512], bf16, tag="rqc", bufs=3, name="rqc")
                        nc.vector.tensor_mul(qc, stg_t[i][sc], cosdr[:, i, csl])
                        nc.vector.tensor_add(dst[ht][:, i, csl], tmp, qc)

            def v_fills(s, xh, prep_p):
                wvf = prep_p.tile([P, 8, NHL * D], f32r, tag="wvf", name="wvf")
                nc.gpsimd.dma_start(out=wvf, in_=wv_d[s][:].rearrange("(c p) n -> p c n", p=P))
                for st in range(8):
                    psv = pp.tile([P, 512], f32, tag="mm", name="psv")
                    for kc in range(8):
                        nc.tensor.matmul(
                            psv,
                            xh[kc][:, st * P:(st + 1) * P],
                            wvf[:, kc, :],
                            start=(kc == 0), stop=(kc == 7),
                        )
                    nc.vector.memset(v_sb[s][st][:, :, D:D + 1], 1.0)
                    nc.vector.tensor_copy(
                        out=v_sb[s][st][:, :, 0:D],
                        in_=psv.rearrange("p (h d) -> p h d", d=D),
                    )

            # ---------------- attention slots (phase-major) ----------------
            pa_t = {}

            def half_attn(acc, s, ht, hh, ts):
                h = 4 * ht + hh
                pr = slice(32 * hh, 32 * hh + 32)
                tpos = (32 * hh, 0)
                for tst in range(8):
                    ps = scps.tile([P, S], f32, tag="sc", name="ps")
                    for sc in range(2):
                        csl = slice(sc * 512, (sc + 1) * 512)
                        nc.tensor.matmul(
                            ps[:, csl],
                            kdr[ts][ht][pr, :, tst * P:(tst + 1) * P],
                            qdr[s][ht][pr, :, csl],
                            perf_mode=DR,
                            tile_position=tpos,
                        )
                    es = es_pool.tile([P, S], bf16, tag="es", bufs=6, name="es")
                    nc.scalar.activation(out=es, in_=ps, func=AF.Exp, bias=zeroc, scale=SCALE)
                    for sc in range(2):
                        csl = slice(sc * 512, (sc + 1) * 512)
                        nc.tensor.matmul(
                            acc[:, csl],
                            v_sb[ts][tst][:, h, :],
                            es[:, csl],
                            start=(tst == 0), stop=(tst == 7),
                        )

            def slot_a(s, ht, hh):
                acc = accps.tile([D + 1, S], f32, tag="acc", name="acc")
                half_attn(acc, s, ht, hh, 0)
                pa = pa_pool.tile([D + 1, S], bf16, tag="pa", bufs=16, name="pa")
                nc.vector.tensor_copy(out=pa, in_=acc)
                pa_t[(s, ht, hh)] = pa

            def slot_b(s, ht, hh):
                h = 4 * ht + hh
                acc = accps.tile([D + 1, S], f32, tag="acc", name="acc")
                half_attn(acc, s, ht, hh, 1)
                nsum = small.tile([D + 1, S], f32, tag="nsum", bufs=2, name="nsum")
                nc.vector.tensor_add(nsum, acc, pa_t[(s, ht, hh)])
                rstg = small.tile([1, S], f32, tag="rstg", bufs=1, name="rstg")
                nc.vector.reciprocal(out=rstg, in_=nsum[D:D + 1, :])
                rbc = small.tile([D, S], f32, tag="rbc", bufs=1, name="rbc")
                nc.gpsimd.partition_broadcast(rbc, rstg)
                nc.vector.tensor_mul(attn[s][h // 2][(h % 2) * 64:(h % 2) * 64 + D, :],
                                     nsum[0:D, :], rbc)

            # ---------------- tail: out-projection per stream ----------------
            def tail(s):
                with tc.tile_pool(name=f"wo{s}", bufs=1) as wop:
                    wo_t = [wop.tile([P, H], f32r, tag=f"wo{p}", name=f"wo{p}") for p in range(4)]
                    for p in range(4):
                        nc.sync.dma_start(out=wo_t[p], in_=wout_d[s][p * P:(p + 1) * P, :])
                    for st in range(8):
                        for oc in range(2):
                            pso = pp.tile([P, 512], f32, tag="mm", name="pso")
                            for p in range(4):
                                nc.tensor.matmul(
                                    pso,
                                    attn[s][p][:, st * P:(st + 1) * P],
                                    wo_t[p][:, oc * 512:(oc + 1) * 512],
                                    start=(p == 0), stop=(p == 3),
                                )
                            osb = small.tile([P, 512], f32, tag="osb", bufs=3, name="osb")
                            if s == 1 and (st + oc) % 2 == 0:
                                nc.scalar.copy(out=osb, in_=pso)
                            else:
                                nc.vector.tensor_copy(out=osb, in_=pso)
                            (nc.gpsimd if (st + oc) % 2 == 0 else nc.sync).dma_start(
                                out=out_d[s][st * P:(st + 1) * P, oc * 512:(oc + 1) * 512], in_=osb)

            # ---------------- emission ----------------
            with tc.tile_pool(name="prep", bufs=1) as prep_p:
                for s in range(2):
                    xh, cq = load_stream(s, prep_p)
                    qk_fills(s, xh, cq, 0, 0, prep_p)
                    qk_fills(s, xh, cq, 1, 0, prep_p)
                    v_fills(s, xh, prep_p)
                    for hh in range(4):
                        slot_a(s, 0, hh)
                    qk_fills(s, xh, cq, 0, 1, prep_p)
                    qk_fills(s, xh, cq, 1, 1, prep_p)
                    for hh in range(4):
                        slot_a(s, 1, hh)
            for (s, ht, hh) in [(s, ht, hh) for s in range(2) for ht in range(2) for hh in range(4)]:
                slot_b(s, ht, hh)
                if (s, ht, hh) == (0, 1, 3):
                    tail(0)
            tail(1)

    if do_compile:
        nc.compile()
    return nc


def _host_prep(x_a, x_b, Wqkv_a, Wqkv_b, Wout_a, Wout_b,
               gamma_a, beta_a, gamma_b, beta_b, height, width):
    """Build the 8 per-core input maps."""
    import ml_dtypes
    cos, sin = _rope_tables(height, width)      # [S, 64]

    # DR rope tables [128, 2, S]: partition p = 32*hh + dl (repeats over hh)
    dl = np.arange(32)
    cos_dr = np.empty((P, 2, S), np.float32)
    sin_dr = np.empty((P, 2, S), np.float32)
    for hh in range(4):
        rows = 32 * hh + dl
        cos_dr[rows, 0, :] = cos[:, dl].T          # cos[s, dl]
        cos_dr[rows, 1, :] = cos[:, 32 + dl].T
        sin_dr[rows, 0, :] = -sin[:, dl].T         # pre-negated for i=0
        sin_dr[rows, 1, :] = sin[:, 32 + dl].T
    cos_dr = np.ascontiguousarray(cos_dr.reshape(P, 2 * S).astype(ml_dtypes.bfloat16))
    sin_dr = np.ascontiguousarray(sin_dr.reshape(P, 2 * S).astype(ml_dtypes.bfloat16))

    # host LayerNorm fold: xhat = r*(x-mu), shipped transposed [H, S]
    def xhat(x):
        x = x.astype(np.float32)
        mu = x.mean(axis=-1, keepdims=True)
        var = ((x - mu) ** 2).mean(axis=-1, keepdims=True)
        r = 1.0 / np.sqrt(var + LN_EPS)
        return ((x - mu) * r).astype(np.float32)

    streams = []
    vshifts = []
    for (W, Wo, g, b) in ((Wqkv_a, Wout_a, gamma_a, beta_a), (Wqkv_b, Wout_b, gamma_b, beta_b)):
        Wg = (W * g[:, None]).astype(np.float32)       # gamma-folded
        cfull = (b.astype(np.float64) @ W.astype(np.float64)).astype(np.float32)  # beta@W [3H]
        W4 = Wg.reshape(H, 3, NH, D)
        c4 = cfull.reshape(3, NH, D)
        per_hg = []
        for hg in range(2):
            h0 = hg * NHL
            # DR column order: blocks (qk, ht, half) of 128 cols = (hh, dl)
            cols = []
            ccols = []
            for qk in range(2):
                for ht in range(2):
                    for half in range(2):
                        for hh in range(4):
                            head = h0 + 4 * ht + hh
                            dsl = slice(32 * half, 32 * half + 32)
                            cols.append(W4[:, qk, head, dsl])      # [H, 32]
                            ccols.append(c4[qk, head, dsl])        # [32]
            wqk = np.ascontiguousarray(np.concatenate(cols, axis=1))       # [H, 1024]
            # beta@W per qk column as per-partition scalars [128, 8 blocks]
            cqk_blk = np.ascontiguousarray(
                np.concatenate(ccols).reshape(8, P).T.astype(np.float32))   # [128, 8]
            wv = np.ascontiguousarray(W4[:, 2, h0:h0 + NHL, :].reshape(H, NHL * D))
            wout = np.ascontiguousarray(Wo.reshape(NH, D, H)[h0:h0 + NHL].reshape(NHL * D, H).astype(np.float32))
            per_hg.append(dict(wqk=wqk, wv=wv, cqk=cqk_blk, wout=wout))
        # exact host-side V correction: beta@Wv shifts attn uniformly
        # (softmax weights sum to 1), so it lands as a constant row on out
        vshift = (cfull[2 * H:3 * H].astype(np.float64) @ Wo.astype(np.float64)).astype(np.float32)
        streams.append(per_hg)
        vshifts.append(vshift)

    in_maps = []
    B = x_a.shape[0]
    xh_a = [np.ascontiguousarray(xhat(x_a[b_i]).T) for b_i in range(B)]
    xh_b = [np.ascontiguousarray(xhat(x_b[b_i]).T) for b_i in range(B)]
    for c in range(N_CORES):
        b_i, hg = (c // 2) % B, c % 2
        m = {
            "xh_s0": xh_a[b_i],
            "xh_s1": xh_b[b_i],
            "cosdr": cos_dr, "sindr": sin_dr,
        }
        for s in range(2):
            blk = streams[s][hg]
            m[f"wqk_s{s}"] = blk["wqk"]
            m[f"wv_s{s}"] = blk["wv"]
            m[f"cqk_s{s}"] = blk["cqk"]
            m[f"wout_s{s}"] = blk["wout"]
        in_maps.append(m)
    return in_maps, vshifts


def kernel(x_a, x_b, Wqkv_a, Wqkv_b, Wout_a, Wout_b,
           gamma_a, beta_a, gamma_b, beta_b, height, width):
    from concourse.bass_utils import run_bass_kernel_spmd

    x_a = np.asarray(x_a, dtype=np.float32)
    x_b = np.asarray(x_b, dtype=np.float32)
    B = x_a.shape[0]
    in_maps, vshifts = _host_prep(x_a, x_b,
                         np.asarray(Wqkv_a, np.float32), np.asarray(Wqkv_b, np.float32),
                         np.asarray(Wout_a, np.float32), np.asarray(Wout_b, np.float32),
                         np.asarray(gamma_a, np.float32), np.asarray(beta_a, np.float32),
                         np.asarray(gamma_b, np.float32), np.asarray(beta_b, np.float32),
                         height, width)
    nc = _get_program()
    res = run_bass_kernel_spmd(nc, in_maps, list(range(N_CORES))).results
    out_a = np.empty((B, S, H), np.float32)
    out_b = np.empty((B, S, H), np.float32)
    for b_i in range(B):
        out_a[b_i] = res[2 * b_i]["out_s0"] + res[2 * b_i + 1]["out_s0"] + vshifts[0]
        out_b[b_i] = res[2 * b_i]["out_s1"] + res[2 * b_i + 1]["out_s1"] + vshifts[1]
    return out_a, out_b


def _get_program():
    global _PROGRAM
    if _PROGRAM is None:
        _PROGRAM = _build_program()
    return _PROGRAM
